# revision 9
# baseline (speedup 1.0000x reference)
"""GAT layer (2 heads) Bass kernel for Trainium2, sharded over 8 NeuronCores.

Computation (per reference):
    Wh   = h_s @ W                      [N, D]
    e_h  = leaky_relu(Wh @ h_k^T)       [N, M]   (alpha = 0.2)
    att  = softmax(where(h_s > 0, e_h, -9e15), axis=-1)
    h'_h = (att * dropout_mask) @ h_k   [N, D]
    out  = elu(h'_1 + h'_2)
Returns (out, att1, att2).

Sharding: rows of h_s (N) split across 8 cores; weights replicated.
Dropout masks are reproduced on host (fixed jax threefry key) and shipped
as {0,1} uint8 in transposed layout; the 1/keep factor and the softmax
normalization are folded into a tiny per-row scale of h'.
"""

import sys
from contextlib import ExitStack

import numpy as np

if "/opt/trn_rl_repo" not in sys.path:
    sys.path.insert(0, "/opt/trn_rl_repo")

import concourse.bass as bass
import concourse.tile as tile
from concourse import bacc, mybir
from concourse.masks import make_identity

F32 = mybir.dt.float32
U8 = mybir.dt.uint8

N_FULL = 8192
M_FULL = 4096
D_FULL = 128
N_CORES = 8
ALPHA = 0.2
RATE = 0.1
KEEP = 1.0 - RATE
NEG_MASK = -1.0e30  # added to masked logits (reference uses -9e15; any << min kept logit works)
NEG_INIT = -3.0e38  # running-max init


def build_gat(nc, R, M, D, use_pool_cvt=False, variant="prelu"):
    """Emit the per-core GAT kernel into `nc` (R rows per core).

    variant:
      "prelu"   — z = e + negoff on PE; ACT Prelu; one big ACT Exp with sum-accum.
      "dualexp" — exp(leaky(z)) = max(exp(z), exp(alpha*z)); DVE stt merges + sums.
    Both skip the softmax max-subtraction: |z| <= |Wh_r||hk_m| < 90 so exp stays
    finite in f32, and normalization divides the scale back out.
    """
    assert R % 128 == 0 and M % 512 == 0 and D == 128
    RB = R // 128   # row blocks
    MC = M // 128   # 128-wide m chunks
    EC = M // 512   # 512-wide e chunks

    op = mybir.AluOpType
    AF = mybir.ActivationFunctionType

    hs = nc.dram_tensor("hs", [R, M], F32, kind="ExternalInput").ap()
    w = nc.dram_tensor("w", [128, MC, D], F32, kind="ExternalInput").ap()
    hk1T = nc.dram_tensor("hk1t", [D, M], F32, kind="ExternalInput").ap()
    hk2T = nc.dram_tensor("hk2t", [D, M], F32, kind="ExternalInput").ap()
    hk1r = nc.dram_tensor("hk1r", [128, MC, D], F32, kind="ExternalInput").ap()
    hk2r = nc.dram_tensor("hk2r", [128, MC, D], F32, kind="ExternalInput").ap()
    m1t = nc.dram_tensor("m1t", [RB, 128, MC, 128], U8, kind="ExternalInput").ap()
    m2t = nc.dram_tensor("m2t", [RB, 128, MC, 128], U8, kind="ExternalInput").ap()
    att1 = nc.dram_tensor("att1", [R, M], F32, kind="ExternalOutput").ap()
    att2 = nc.dram_tensor("att2", [R, M], F32, kind="ExternalOutput").ap()
    outp = nc.dram_tensor("outp", [R, D], F32, kind="ExternalOutput").ap()

    with tile.TileContext(nc) as tc, ExitStack() as ctx:
        const = ctx.enter_context(tc.tile_pool(name="const", bufs=1))
        hs_pool = ctx.enter_context(tc.tile_pool(name="hs", bufs=2))
        hsT_pool = ctx.enter_context(tc.tile_pool(name="hsT", bufs=3))
        negoff_pool = ctx.enter_context(tc.tile_pool(name="negoff", bufs=1))
        eab_pool = ctx.enter_context(tc.tile_pool(name="eab", bufs=3))
        em_pool = ctx.enter_context(tc.tile_pool(name="em", bufs=1))
        ex_pool = ctx.enter_context(tc.tile_pool(name="ex", bufs=2))
        mu8_pool = ctx.enter_context(tc.tile_pool(name="mu8", bufs=2))
        mf_pool = ctx.enter_context(tc.tile_pool(name="mf", bufs=3))
        attmT_pool = ctx.enter_context(tc.tile_pool(name="attmT", bufs=3))
        whT_pool = ctx.enter_context(tc.tile_pool(name="whT", bufs=2))
        small = ctx.enter_context(tc.tile_pool(name="small", bufs=4))
        hp_pool = ctx.enter_context(tc.tile_pool(name="hp", bufs=2))

        psum_t = ctx.enter_context(tc.tile_pool(name="psum_t", bufs=2, space="PSUM"))
        psum_wh = ctx.enter_context(tc.tile_pool(name="psum_wh", bufs=1, space="PSUM"))
        psum_e = ctx.enter_context(tc.tile_pool(name="psum_e", bufs=2, space="PSUM"))
        psum_hp = ctx.enter_context(tc.tile_pool(name="psum_hp", bufs=2, space="PSUM"))

        ident = const.tile([128, 128], F32)
        make_identity(nc, ident)

        w_sb = const.tile([128, MC, D], F32)
        nc.sync.dma_start(out=w_sb, in_=w)
        hk1T_sb = const.tile([D, M], F32)
        nc.sync.dma_start(out=hk1T_sb, in_=hk1T)
        hk2T_sb = const.tile([D, M], F32)
        nc.sync.dma_start(out=hk2T_sb, in_=hk2T)
        hk1r_sb = const.tile([128, MC, D], F32)
        nc.sync.dma_start(out=hk1r_sb, in_=hk1r)
        hk2r_sb = const.tile([128, MC, D], F32)
        nc.sync.dma_start(out=hk2r_sb, in_=hk2r)

        for b in range(RB):
            hs_sb = hs_pool.tile([128, M], F32, tag="hs")
            nc.sync.dma_start(out=hs_sb, in_=hs[b * 128 : (b + 1) * 128, :])

            # h_s^T tiles (PE transpose) feeding WhT = W^T @ h_s^T accumulation
            wh_ps = psum_wh.tile([D, 128], F32, tag="wh")
            for g in range(EC):
                t_ps = psum_t.tile([128, 4, 128], F32, tag="tps")
                for jj in range(4):
                    j = g * 4 + jj
                    nc.tensor.transpose(
                        t_ps[:, jj, :], hs_sb[:, j * 128 : (j + 1) * 128], ident
                    )
                hsT_sb = hsT_pool.tile([128, 4, 128], F32, tag="hsT")
                nc.scalar.copy(hsT_sb, t_ps)
                for jj in range(4):
                    j = g * 4 + jj
                    nc.tensor.matmul(
                        wh_ps,
                        lhsT=w_sb[:, j, :],
                        rhs=hsT_sb[:, jj, :],
                        start=(j == 0),
                        stop=(j == MC - 1),
                    )
            whT_sb = whT_pool.tile([D, 128], F32, tag="whT")
            nc.scalar.copy(whT_sb, wh_ps)

            # negoff = (h_s <= 0) * NEG_MASK   {0 kept, NEG masked}
            negoff = negoff_pool.tile([128, M], F32, tag="negoff")
            nc.vector.tensor_scalar(
                out=negoff,
                in0=hs_sb,
                scalar1=0.0,
                scalar2=NEG_MASK,
                op0=op.is_le,
                op1=op.mult,
            )

            hp_sbs = []
            for h, (hkT_sb, hkr_sb, mt, att) in enumerate(
                ((hk1T_sb, hk1r_sb, m1t, att1), (hk2T_sb, hk2r_sb, m2t, att2))
            ):
                m_u8 = mu8_pool.tile([128, MC, 128], U8, tag="mu8")
                nc.sync.dma_start(out=m_u8, in_=mt[b])

                ex = ex_pool.tile([128, M], F32, tag="ex")
                ssum = small.tile([128, 1], F32, tag="ssum")
                if variant == "prelu":
                    em = em_pool.tile([128, M], F32, tag="em")
                    for c in range(EC):
                        sl = slice(c * 512, (c + 1) * 512)
                        e_ps = psum_e.tile([128, 512], F32, tag="eps")
                        # z = negoff + Wh @ hk^T  (mask-add seeded via identity matmul)
                        nc.tensor.matmul(
                            e_ps, lhsT=ident, rhs=negoff[:, sl], start=True, stop=False
                        )
                        nc.tensor.matmul(
                            e_ps, lhsT=whT_sb, rhs=hkT_sb[:, sl], start=False, stop=True
                        )
                        nc.scalar.activation(em[:, sl], e_ps, AF.Prelu, alpha=ALPHA)
                    # softmax without max-subtraction: exp(z) is safely finite
                    nc.scalar.activation(ex, em, AF.Exp, accum_out=ssum)
                else:  # dualexp
                    sums = small.tile([128, EC], F32, tag="sums")
                    for c in range(EC):
                        sl = slice(c * 512, (c + 1) * 512)
                        e_ps = psum_e.tile([128, 512], F32, tag="eps")
                        nc.tensor.matmul(
                            e_ps, lhsT=ident, rhs=negoff[:, sl], start=True, stop=False
                        )
                        nc.tensor.matmul(
                            e_ps, lhsT=whT_sb, rhs=hkT_sb[:, sl], start=False, stop=True
                        )
                        exa = eab_pool.tile([128, 512], F32, tag="exa")
                        nc.scalar.activation(exa, e_ps, AF.Exp)
                        exb = eab_pool.tile([128, 512], F32, tag="exb")
                        nc.scalar.activation(exb, e_ps, AF.Exp, scale=ALPHA)
                        # exp(leaky(z)) = max(exp(z), exp(alpha z)); accum = row sum
                        nc.vector.scalar_tensor_tensor(
                            out=ex[:, sl], in0=exa, scalar=0.0, op0=op.add,
                            op1=op.max, in1=exb, accum_out=sums[:, c : c + 1],
                        )
                    nc.vector.tensor_reduce(
                        ssum, sums, axis=mybir.AxisListType.X, op=op.add
                    )
                rinv = small.tile([128, 1], F32, tag="rinv")
                nc.vector.reciprocal(rinv, ssum)
                rinvk = small.tile([128, 1], F32, tag="rinvk")
                nc.vector.tensor_scalar_mul(rinvk, rinv, 1.0 / KEEP)

                hp_ps = psum_hp.tile([128, D], F32, tag="hp")
                for g in range(EC):
                    t_ps = psum_t.tile([128, 4, 128], F32, tag="tps")
                    for jj in range(4):
                        j = g * 4 + jj
                        nc.tensor.transpose(
                            t_ps[:, jj, :], ex[:, j * 128 : (j + 1) * 128], ident
                        )
                    attmT = attmT_pool.tile([128, 4, 128], F32, tag="attmT")
                    if use_pool_cvt:
                        mf = mf_pool.tile([128, 4, 128], F32, tag="mf")
                        nc.gpsimd.tensor_copy(out=mf, in_=m_u8[:, g * 4 : (g + 1) * 4, :])
                        nc.vector.tensor_mul(attmT, t_ps, mf)
                    else:
                        nc.vector.tensor_mul(attmT, t_ps, m_u8[:, g * 4 : (g + 1) * 4, :])
                    for jj in range(4):
                        j = g * 4 + jj
                        nc.tensor.matmul(
                            hp_ps,
                            lhsT=attmT[:, jj, :],
                            rhs=hkr_sb[:, j, :],
                            start=(j == 0),
                            stop=(j == MC - 1),
                        )

                # att output = ex * (1/sum), in place, then store
                nc.vector.tensor_scalar_mul(ex, ex, rinv)
                nc.sync.dma_start(out=att[b * 128 : (b + 1) * 128, :], in_=ex)

                hp_sb = hp_pool.tile([128, D], F32, tag=f"hp{h}")
                nc.vector.tensor_scalar_mul(hp_sb, hp_ps, rinvk)
                hp_sbs.append(hp_sb)

            # out = elu(hp1 + hp2) = max(s,0) + exp(min(s,0)) - 1
            s = hp_pool.tile([128, D], F32, tag="hsum")
            nc.vector.tensor_add(s, hp_sbs[0], hp_sbs[1])
            sneg = hp_pool.tile([128, D], F32, tag="hneg")
            nc.vector.tensor_scalar_min(sneg, s, 0.0)
            spos = hp_pool.tile([128, D], F32, tag="hpos")
            nc.vector.tensor_scalar_max(spos, s, 0.0)
            ev = hp_pool.tile([128, D], F32, tag="hev")
            nc.scalar.activation(ev, sneg, AF.Exp)
            o = hp_pool.tile([128, D], F32, tag="hout")
            nc.vector.scalar_tensor_tensor(
                out=o, in0=ev, scalar=-1.0, op0=op.add, op1=op.add, in1=spos
            )
            nc.sync.dma_start(out=outp[b * 128 : (b + 1) * 128, :], in_=o)

    return nc


def _arrange_weight(a):
    """[M, D] -> [128, M//128, D] with partition = m % 128 (within chunk)."""
    m, d = a.shape
    return np.ascontiguousarray(a.reshape(m // 128, 128, d).transpose(1, 0, 2))


def _arrange_maskT(mask_rows):
    """{0,1} mask [R, M] -> [R//128, 128, M//128, 128] u8, [b, p=m%128, c=m//128, r%128]."""
    r, m = mask_rows.shape
    a = mask_rows.reshape(r // 128, 128, m // 128, 128)  # [b, r, c, p]
    return np.ascontiguousarray(a.transpose(0, 3, 2, 1)).astype(np.uint8)


def _dropout_masks():
    """Reproduce the reference's bernoulli keep-masks ({0,1}) on host CPU."""
    import jax

    cpu = jax.devices("cpu")[0]
    with jax.default_device(cpu):
        dk1, dk2 = jax.random.split(jax.random.key(1234))
        m1 = jax.random.bernoulli(dk1, KEEP, (N_FULL, M_FULL))
        m2 = jax.random.bernoulli(dk2, KEEP, (N_FULL, M_FULL))
        return np.asarray(m1), np.asarray(m2)


_BUILT = {}


def _get_nc(R=N_FULL // N_CORES, M=M_FULL, D=D_FULL, variant="prelu"):
    key = (R, M, D, variant)
    if key not in _BUILT:
        nc = bacc.Bacc("TRN2", target_bir_lowering=False, debug=False,
                       num_devices=N_CORES)
        build_gat(nc, R, M, D, variant=variant)
        nc.compile()
        _BUILT[key] = nc
    return _BUILT[key]


def _make_in_maps(h_s, h_k1, h_k2, W):
    h_s = np.ascontiguousarray(h_s, dtype=np.float32)
    h_k1 = np.ascontiguousarray(h_k1, dtype=np.float32)
    h_k2 = np.ascontiguousarray(h_k2, dtype=np.float32)
    W = np.ascontiguousarray(W, dtype=np.float32)

    m1, m2 = _dropout_masks()
    R = N_FULL // N_CORES

    w_arr = _arrange_weight(W)
    hk1T = np.ascontiguousarray(h_k1.T)
    hk2T = np.ascontiguousarray(h_k2.T)
    hk1r = _arrange_weight(h_k1)
    hk2r = _arrange_weight(h_k2)

    in_maps = []
    for i in range(N_CORES):
        rows = slice(i * R, (i + 1) * R)
        in_maps.append(
            {
                "hs": np.ascontiguousarray(h_s[rows]),
                "w": w_arr,
                "hk1t": hk1T,
                "hk2t": hk2T,
                "hk1r": hk1r,
                "hk2r": hk2r,
                "m1t": _arrange_maskT(m1[rows]),
                "m2t": _arrange_maskT(m2[rows]),
            }
        )
    return in_maps


def _gather(res):
    out = np.concatenate([r["outp"] for r in res], axis=0)
    att1 = np.concatenate([r["att1"] for r in res], axis=0)
    att2 = np.concatenate([r["att2"] for r in res], axis=0)
    return out, att1, att2


def kernel(h_s, h_k1, h_k2, W):
    from concourse.bass_utils import run_bass_kernel_spmd

    in_maps = _make_in_maps(h_s, h_k1, h_k2, W)
    nc = _get_nc()
    res = run_bass_kernel_spmd(nc, in_maps, list(range(N_CORES))).results
    return _gather(res)


# revision 18
# speedup vs baseline: 1.4746x; 1.4746x over previous
"""GAT layer (2 heads) Bass kernel for Trainium2, sharded over 8 NeuronCores.

Computation (per reference):
    Wh   = h_s @ W                      [N, D]
    e_h  = leaky_relu(Wh @ h_k^T)       [N, M]   (alpha = 0.2)
    att  = softmax(where(h_s > 0, e_h, -9e15), axis=-1)
    h'_h = (att * dropout_mask) @ h_k   [N, D]
    out  = elu(h'_1 + h'_2)
Returns (out, att1, att2).

Sharding: rows of h_s (N) split across 8 cores; weights replicated.
Dropout masks are reproduced on host (fixed jax threefry key) and shipped
as {0,1} uint8 in transposed layout; the 1/keep factor and the softmax
normalization are folded into a tiny per-row scale of h'.
"""

import sys
from contextlib import ExitStack

import numpy as np

if "/opt/trn_rl_repo" not in sys.path:
    sys.path.insert(0, "/opt/trn_rl_repo")

import concourse.bass as bass
import concourse.tile as tile
from concourse import bacc, mybir
from concourse.masks import make_identity

F32 = mybir.dt.float32
BF16 = mybir.dt.bfloat16
U8 = mybir.dt.uint8

N_FULL = 8192
M_FULL = 4096
D_FULL = 128
N_CORES = 8
ALPHA = 0.2
RATE = 0.1
KEEP = 1.0 - RATE
NEG_MASK = -1.0e30  # added to masked logits (reference uses -9e15; any << min kept logit works)
NEG_INIT = -3.0e38  # running-max init


def build_gat(nc, R, M, D, use_pool_cvt=False, variant="prelu"):
    """Emit the per-core GAT kernel into `nc` (R rows per core).

    variant:
      "prelu"   — z = e + negoff on PE; ACT Prelu; one big ACT Exp with sum-accum.
      "dualexp" — exp(leaky(z)) = max(exp(z), exp(alpha*z)); DVE stt merges + sums.
    Both skip the softmax max-subtraction: |z| <= |Wh_r||hk_m| < 90 so exp stays
    finite in f32, and normalization divides the scale back out.
    """
    assert R % 128 == 0 and M % 512 == 0 and D == 128
    RB = R // 128   # row blocks
    MC = M // 128   # 128-wide m chunks
    EC = M // 512   # 512-wide e chunks

    op = mybir.AluOpType
    AF = mybir.ActivationFunctionType

    hst = nc.dram_tensor("hst", [RB, 128, MC, 128], F32, kind="ExternalInput").ap()
    m01 = nc.dram_tensor("m01", [RB, 128, M], U8, kind="ExternalInput").ap()
    w = nc.dram_tensor("w", [128, MC, D], F32, kind="ExternalInput").ap()
    hk1T = nc.dram_tensor("hk1t", [D, M], F32, kind="ExternalInput").ap()
    hk2T = nc.dram_tensor("hk2t", [D, M], F32, kind="ExternalInput").ap()
    hk1r = nc.dram_tensor("hk1r", [128, MC, D], BF16, kind="ExternalInput").ap()
    hk2r = nc.dram_tensor("hk2r", [128, MC, D], BF16, kind="ExternalInput").ap()
    m1t = nc.dram_tensor("m1t", [RB, 128, MC, 128], U8, kind="ExternalInput").ap()
    m2t = nc.dram_tensor("m2t", [RB, 128, MC, 128], U8, kind="ExternalInput").ap()
    att1 = nc.dram_tensor("att1", [R, M], F32, kind="ExternalOutput").ap()
    att2 = nc.dram_tensor("att2", [R, M], F32, kind="ExternalOutput").ap()
    outp = nc.dram_tensor("outp", [R, D], F32, kind="ExternalOutput").ap()

    with tile.TileContext(nc) as tc, ExitStack() as ctx:
        const = ctx.enter_context(tc.tile_pool(name="const", bufs=1))
        hs_pool = ctx.enter_context(tc.tile_pool(name="hs", bufs=2))
        hsT_pool = ctx.enter_context(tc.tile_pool(name="hsT", bufs=3))
        negoff_pool = ctx.enter_context(tc.tile_pool(name="negoff", bufs=1))
        eab_pool = ctx.enter_context(tc.tile_pool(name="eab", bufs=3))
        em_pool = ctx.enter_context(tc.tile_pool(name="em", bufs=1))
        ex_pool = ctx.enter_context(tc.tile_pool(name="ex", bufs=2))
        mu8_pool = ctx.enter_context(tc.tile_pool(name="mu8", bufs=2))
        mf_pool = ctx.enter_context(tc.tile_pool(name="mf", bufs=3))
        attmT_pool = ctx.enter_context(tc.tile_pool(name="attmT", bufs=3))
        whT_pool = ctx.enter_context(tc.tile_pool(name="whT", bufs=2))
        small = ctx.enter_context(tc.tile_pool(name="small", bufs=4))
        hp_pool = ctx.enter_context(tc.tile_pool(name="hp", bufs=2))

        psum_t = ctx.enter_context(tc.tile_pool(name="psum_t", bufs=2, space="PSUM"))
        psum_wh = ctx.enter_context(tc.tile_pool(name="psum_wh", bufs=1, space="PSUM"))
        psum_e = ctx.enter_context(tc.tile_pool(name="psum_e", bufs=4, space="PSUM"))
        psum_hp = ctx.enter_context(tc.tile_pool(name="psum_hp", bufs=1, space="PSUM"))

        ident = const.tile([128, 128], F32)
        make_identity(nc, ident)

        w_sb = const.tile([128, MC, D], F32)
        nc.sync.dma_start(out=w_sb, in_=w)
        hk1T_sb = const.tile([D, M], F32)
        nc.sync.dma_start(out=hk1T_sb, in_=hk1T)
        hk2T_sb = const.tile([D, M], F32)
        nc.sync.dma_start(out=hk2T_sb, in_=hk2T)
        hk1r_sb = const.tile([128, MC, D], BF16)
        nc.sync.dma_start(out=hk1r_sb, in_=hk1r)
        hk2r_sb = const.tile([128, MC, D], BF16)
        nc.sync.dma_start(out=hk2r_sb, in_=hk2r)

        for b in range(RB):
            # h_s^T tiles come pre-transposed from the host
            hsT_sb = hs_pool.tile([128, MC, 128], F32, tag="hst")
            nc.sync.dma_start(out=hsT_sb, in_=hst[b])
            m01_sb = mu8_pool.tile([128, M], U8, tag="m01")
            nc.sync.dma_start(out=m01_sb, in_=m01[b])

            # WhT[d, r] = sum_j W_j^T @ hsT_j
            wh_ps = psum_wh.tile([D, 128], F32, tag="wh")
            for j in range(MC):
                nc.tensor.matmul(
                    wh_ps,
                    lhsT=w_sb[:, j, :],
                    rhs=hsT_sb[:, j, :],
                    start=(j == 0),
                    stop=(j == MC - 1),
                )
            whT_sb = whT_pool.tile([D, 128], F32, tag="whT")
            nc.scalar.copy(whT_sb, wh_ps)

            # negoff = (h_s <= 0) * NEG_MASK   {0 kept, NEG masked}
            negoff = negoff_pool.tile([128, M], F32, tag="negoff")
            nc.vector.tensor_scalar(
                out=negoff,
                in0=m01_sb,
                scalar1=0,
                scalar2=NEG_MASK,
                op0=op.is_equal,
                op1=op.mult,
            )

            hp_sbs = []
            for h, (hkT_sb, hkr_sb, mt, att) in enumerate(
                ((hk1T_sb, hk1r_sb, m1t, att1), (hk2T_sb, hk2r_sb, m2t, att2))
            ):
                m_u8 = mu8_pool.tile([128, MC, 128], U8, tag="mu8")
                nc.sync.dma_start(out=m_u8, in_=mt[b])

                ex = ex_pool.tile([128, M], F32, tag="ex")
                ssum = small.tile([128, 1], F32, tag="ssum")
                if variant == "prelu":
                    GRP = min(4, EC)
                    em = em_pool.tile([128, M], F32, tag="em")
                    for g in range(EC // GRP):
                        # batch chunks so same-weight matmuls run back to back
                        e_pss = []
                        for cc in range(GRP):
                            c = g * GRP + cc
                            sl = slice(c * 512, (c + 1) * 512)
                            e_ps = psum_e.tile([128, 512], F32, tag="eps")
                            e_pss.append((e_ps, sl))
                            # z = negoff + Wh @ hk^T (mask-add seeded via identity)
                            nc.tensor.matmul(
                                e_ps, lhsT=ident, rhs=negoff[:, sl],
                                start=True, stop=False,
                            )
                        for e_ps, sl in e_pss:
                            nc.tensor.matmul(
                                e_ps, lhsT=whT_sb, rhs=hkT_sb[:, sl],
                                start=False, stop=True,
                            )
                        for e_ps, sl in e_pss:
                            nc.scalar.activation(em[:, sl], e_ps, AF.Prelu, alpha=ALPHA)
                    # softmax without max-subtraction: exp(z) is safely finite
                    nc.scalar.activation(ex, em, AF.Exp, accum_out=ssum)
                else:  # dualexp
                    sums = small.tile([128, EC], F32, tag="sums")
                    for c in range(EC):
                        sl = slice(c * 512, (c + 1) * 512)
                        e_ps = psum_e.tile([128, 512], F32, tag="eps")
                        nc.tensor.matmul(
                            e_ps, lhsT=ident, rhs=negoff[:, sl], start=True, stop=False
                        )
                        nc.tensor.matmul(
                            e_ps, lhsT=whT_sb, rhs=hkT_sb[:, sl], start=False, stop=True
                        )
                        exa = eab_pool.tile([128, 512], F32, tag="exa")
                        nc.scalar.activation(exa, e_ps, AF.Exp)
                        exb = eab_pool.tile([128, 512], F32, tag="exb")
                        nc.scalar.activation(exb, e_ps, AF.Exp, scale=ALPHA)
                        # exp(leaky(z)) = max(exp(z), exp(alpha z)); accum = row sum
                        nc.vector.scalar_tensor_tensor(
                            out=ex[:, sl], in0=exa, scalar=0.0, op0=op.add,
                            op1=op.max, in1=exb, accum_out=sums[:, c : c + 1],
                        )
                    nc.vector.tensor_reduce(
                        ssum, sums, axis=mybir.AxisListType.X, op=op.add
                    )
                rinv = small.tile([128, 1], F32, tag="rinv")
                nc.vector.reciprocal(rinv, ssum)
                rinvk = small.tile([128, 1], F32, tag="rinvk")
                nc.vector.tensor_scalar_mul(rinvk, rinv, 1.0 / KEEP)

                hp_ps = psum_hp.tile([128, D], F32, tag="hp")
                for g in range(EC):
                    t_ps = psum_t.tile([128, 4, 128], F32, tag="tps")
                    for jj in range(4):
                        j = g * 4 + jj
                        nc.tensor.transpose(
                            t_ps[:, jj, :], ex[:, j * 128 : (j + 1) * 128], ident
                        )
                    attmT = attmT_pool.tile([128, 4, 128], BF16, tag="attmT")
                    nc.vector.tensor_mul(attmT, t_ps, m_u8[:, g * 4 : (g + 1) * 4, :])
                    for jj in range(4):
                        j = g * 4 + jj
                        nc.tensor.matmul(
                            hp_ps,
                            lhsT=attmT[:, jj, :],
                            rhs=hkr_sb[:, j, :],
                            start=(j == 0),
                            stop=(j == MC - 1),
                        )

                # att output = ex * (1/sum), in place, then store
                nc.vector.tensor_scalar_mul(ex, ex, rinv)
                nc.sync.dma_start(out=att[b * 128 : (b + 1) * 128, :], in_=ex)

                hp_sb = hp_pool.tile([128, D], F32, tag=f"hp{h}")
                nc.vector.tensor_scalar_mul(hp_sb, hp_ps, rinvk)
                hp_sbs.append(hp_sb)

            # out = elu(hp1 + hp2) = max(s,0) + exp(min(s,0)) - 1
            s = hp_pool.tile([128, D], F32, tag="hsum")
            nc.vector.tensor_add(s, hp_sbs[0], hp_sbs[1])
            sneg = hp_pool.tile([128, D], F32, tag="hneg")
            nc.vector.tensor_scalar_min(sneg, s, 0.0)
            spos = hp_pool.tile([128, D], F32, tag="hpos")
            nc.vector.tensor_scalar_max(spos, s, 0.0)
            ev = hp_pool.tile([128, D], F32, tag="hev")
            nc.scalar.activation(ev, sneg, AF.Exp)
            o = hp_pool.tile([128, D], F32, tag="hout")
            nc.vector.scalar_tensor_tensor(
                out=o, in0=ev, scalar=-1.0, op0=op.add, op1=op.add, in1=spos
            )
            nc.sync.dma_start(out=outp[b * 128 : (b + 1) * 128, :], in_=o)

    return nc


def _arrange_weight(a, dtype=np.float32):
    """[M, D] -> [128, M//128, D] with partition = m % 128 (within chunk)."""
    m, d = a.shape
    return np.ascontiguousarray(
        a.reshape(m // 128, 128, d).transpose(1, 0, 2).astype(dtype)
    )


def _arrange_T(rows, dtype):
    """[R, M] -> [R//128, 128, M//128, 128] with [b, p=m%128, c=m//128, r%128]."""
    r, m = rows.shape
    a = rows.reshape(r // 128, 128, m // 128, 128)  # [b, r, c, p]
    return np.ascontiguousarray(a.transpose(0, 3, 2, 1).astype(dtype))


def _arrange_maskT(mask_rows):
    return _arrange_T(mask_rows, np.uint8)


def _dropout_masks():
    """Reproduce the reference's bernoulli keep-masks ({0,1}) on host CPU."""
    import jax

    cpu = jax.devices("cpu")[0]
    with jax.default_device(cpu):
        dk1, dk2 = jax.random.split(jax.random.key(1234))
        m1 = jax.random.bernoulli(dk1, KEEP, (N_FULL, M_FULL))
        m2 = jax.random.bernoulli(dk2, KEEP, (N_FULL, M_FULL))
        return np.asarray(m1), np.asarray(m2)


_BUILT = {}


def _get_nc(R=N_FULL // N_CORES, M=M_FULL, D=D_FULL, variant="prelu"):
    key = (R, M, D, variant)
    if key not in _BUILT:
        nc = bacc.Bacc("TRN2", target_bir_lowering=False, debug=False,
                       num_devices=N_CORES)
        build_gat(nc, R, M, D, variant=variant)
        nc.compile()
        _BUILT[key] = nc
    return _BUILT[key]


def _make_in_maps(h_s, h_k1, h_k2, W):
    h_s = np.ascontiguousarray(h_s, dtype=np.float32)
    h_k1 = np.ascontiguousarray(h_k1, dtype=np.float32)
    h_k2 = np.ascontiguousarray(h_k2, dtype=np.float32)
    W = np.ascontiguousarray(W, dtype=np.float32)

    import ml_dtypes

    m1, m2 = _dropout_masks()
    R = N_FULL // N_CORES

    w_arr = _arrange_weight(W)
    hk1T = np.ascontiguousarray(h_k1.T)
    hk2T = np.ascontiguousarray(h_k2.T)
    hk1r = _arrange_weight(h_k1, ml_dtypes.bfloat16)
    hk2r = _arrange_weight(h_k2, ml_dtypes.bfloat16)

    in_maps = []
    for i in range(N_CORES):
        rows = slice(i * R, (i + 1) * R)
        hs_rows = h_s[rows]
        in_maps.append(
            {
                "hst": _arrange_T(hs_rows, np.float32),
                "m01": np.ascontiguousarray(
                    (hs_rows > 0).reshape(R // 128, 128, M_FULL).astype(np.uint8)
                ),
                "w": w_arr,
                "hk1t": hk1T,
                "hk2t": hk2T,
                "hk1r": hk1r,
                "hk2r": hk2r,
                "m1t": _arrange_maskT(m1[rows]),
                "m2t": _arrange_maskT(m2[rows]),
            }
        )
    return in_maps


def _gather(res):
    out = np.concatenate([r["outp"] for r in res], axis=0)
    att1 = np.concatenate([r["att1"] for r in res], axis=0)
    att2 = np.concatenate([r["att2"] for r in res], axis=0)
    return out, att1, att2


def kernel(h_s, h_k1, h_k2, W):
    from concourse.bass_utils import run_bass_kernel_spmd

    in_maps = _make_in_maps(h_s, h_k1, h_k2, W)
    nc = _get_nc()
    res = run_bass_kernel_spmd(nc, in_maps, list(range(N_CORES))).results
    return _gather(res)


# revision 24
# speedup vs baseline: 1.8977x; 1.2869x over previous
"""GAT layer (2 heads) Bass kernel for Trainium2, sharded over 8 NeuronCores.

Computation (per reference):
    Wh   = h_s @ W                      [N, D]
    e_h  = leaky_relu(Wh @ h_k^T)       [N, M]   (alpha = 0.2)
    att  = softmax(where(h_s > 0, e_h, -9e15), axis=-1)
    h'_h = (att * dropout_mask) @ h_k   [N, D]
    out  = elu(h'_1 + h'_2)
Returns (out, att1, att2).

Sharding: rows of h_s (N) split across 8 cores; weights replicated.
Dropout masks are reproduced on host (fixed jax threefry key) and shipped
as {0,1} uint8 in transposed layout; the 1/keep factor and the softmax
normalization are folded into a tiny per-row scale of h'.
"""

import sys
from contextlib import ExitStack

import numpy as np

if "/opt/trn_rl_repo" not in sys.path:
    sys.path.insert(0, "/opt/trn_rl_repo")

import concourse.bass as bass
import concourse.tile as tile
from concourse import bacc, mybir
from concourse.masks import make_identity

F32 = mybir.dt.float32
BF16 = mybir.dt.bfloat16
U8 = mybir.dt.uint8

N_FULL = 8192
M_FULL = 4096
D_FULL = 128
N_CORES = 8
ALPHA = 0.2
RATE = 0.1
KEEP = 1.0 - RATE
NEG_MASK = -1.0e30  # added to masked logits (reference uses -9e15; any << min kept logit works)
NEG_INIT = -3.0e38  # running-max init


def build_gat(nc, R, M, D, use_pool_cvt=False, variant="prelu", use_f32r=True):
    """Emit the per-core GAT kernel into `nc` (R rows per core).

    variant:
      "prelu"   — z = e + negoff on PE; ACT Prelu; one big ACT Exp with sum-accum.
      "dualexp" — exp(leaky(z)) = max(exp(z), exp(alpha*z)); DVE stt merges + sums.
    Both skip the softmax max-subtraction: |z| <= |Wh_r||hk_m| < 90 so exp stays
    finite in f32, and normalization divides the scale back out.
    """
    assert R % 128 == 0 and M % 512 == 0 and D == 128
    RB = R // 128   # row blocks
    MC = M // 128   # 128-wide m chunks
    EC = M // 512   # 512-wide e chunks

    op = mybir.AluOpType
    AF = mybir.ActivationFunctionType
    # fp32 matmuls run LOW+HIGH passes at 2 cyc/col (4x bf16 cost);
    # float32r is single-pass with tf32-class mantissa — fine for logits.
    MMDT = mybir.dt.float32r if use_f32r else F32

    hst = nc.dram_tensor("hst", [RB, 128, MC, 128], MMDT, kind="ExternalInput").ap()
    m01 = nc.dram_tensor("m01", [RB, 128, M], U8, kind="ExternalInput").ap()
    w = nc.dram_tensor("w", [128, MC, D], MMDT, kind="ExternalInput").ap()
    hk1T = nc.dram_tensor("hk1t", [D, M], MMDT, kind="ExternalInput").ap()
    hk2T = nc.dram_tensor("hk2t", [D, M], MMDT, kind="ExternalInput").ap()
    hk1r = nc.dram_tensor("hk1r", [128, MC, D], BF16, kind="ExternalInput").ap()
    hk2r = nc.dram_tensor("hk2r", [128, MC, D], BF16, kind="ExternalInput").ap()
    m1t = nc.dram_tensor("m1t", [RB, 128, MC, 128], U8, kind="ExternalInput").ap()
    m2t = nc.dram_tensor("m2t", [RB, 128, MC, 128], U8, kind="ExternalInput").ap()
    att1 = nc.dram_tensor("att1", [R, M], F32, kind="ExternalOutput").ap()
    att2 = nc.dram_tensor("att2", [R, M], F32, kind="ExternalOutput").ap()
    outp = nc.dram_tensor("outp", [R, D], F32, kind="ExternalOutput").ap()

    with tile.TileContext(nc) as tc, ExitStack() as ctx:
        const = ctx.enter_context(tc.tile_pool(name="const", bufs=1))
        hs_pool = ctx.enter_context(tc.tile_pool(name="hs", bufs=2))
        hsT_pool = ctx.enter_context(tc.tile_pool(name="hsT", bufs=3))
        negoff_pool = ctx.enter_context(tc.tile_pool(name="negoff", bufs=1))
        eab_pool = ctx.enter_context(tc.tile_pool(name="eab", bufs=3))
        em_pool = ctx.enter_context(tc.tile_pool(name="em", bufs=1))
        ex_pool = ctx.enter_context(tc.tile_pool(name="ex", bufs=2))
        mu8_pool = ctx.enter_context(tc.tile_pool(name="mu8", bufs=2))
        mf_pool = ctx.enter_context(tc.tile_pool(name="mf", bufs=3))
        attmT_pool = ctx.enter_context(tc.tile_pool(name="attmT", bufs=3))
        whT_pool = ctx.enter_context(tc.tile_pool(name="whT", bufs=2))
        small = ctx.enter_context(tc.tile_pool(name="small", bufs=4))
        hp_pool = ctx.enter_context(tc.tile_pool(name="hp", bufs=2))

        psum_t = ctx.enter_context(tc.tile_pool(name="psum_t", bufs=2, space="PSUM"))
        psum_wh = ctx.enter_context(tc.tile_pool(name="psum_wh", bufs=1, space="PSUM"))
        psum_e = ctx.enter_context(tc.tile_pool(name="psum_e", bufs=4, space="PSUM"))
        psum_hp = ctx.enter_context(tc.tile_pool(name="psum_hp", bufs=1, space="PSUM"))

        ident = const.tile([128, 128], F32)
        make_identity(nc, ident)

        w_sb = const.tile([128, MC, D], MMDT)
        nc.sync.dma_start(out=w_sb, in_=w)
        hk1T_sb = const.tile([D, M], MMDT)
        nc.sync.dma_start(out=hk1T_sb, in_=hk1T)
        hk2T_sb = const.tile([D, M], MMDT)
        nc.sync.dma_start(out=hk2T_sb, in_=hk2T)
        hk1r_sb = const.tile([128, MC, D], BF16)
        nc.sync.dma_start(out=hk1r_sb, in_=hk1r)
        hk2r_sb = const.tile([128, MC, D], BF16)
        nc.sync.dma_start(out=hk2r_sb, in_=hk2r)

        for b in range(RB):
            # h_s^T tiles come pre-transposed from the host
            hsT_sb = hs_pool.tile([128, MC, 128], MMDT, tag="hst")
            nc.sync.dma_start(out=hsT_sb, in_=hst[b])
            m01_sb = mu8_pool.tile([128, M], U8, tag="m01")
            nc.sync.dma_start(out=m01_sb, in_=m01[b])

            # WhT[d, r] = sum_j W_j^T @ hsT_j
            wh_ps = psum_wh.tile([D, 128], F32, tag="wh")
            for j in range(MC):
                nc.tensor.matmul(
                    wh_ps,
                    lhsT=w_sb[:, j, :],
                    rhs=hsT_sb[:, j, :],
                    start=(j == 0),
                    stop=(j == MC - 1),
                )
            whT_sb = whT_pool.tile([D, 128], MMDT, tag="whT")
            nc.scalar.copy(whT_sb, wh_ps)

            # negoff = (h_s <= 0) * NEG_MASK   {0 kept, NEG masked}
            negoff = negoff_pool.tile([128, M], F32, tag="negoff")
            nc.vector.tensor_scalar(
                out=negoff,
                in0=m01_sb,
                scalar1=0,
                scalar2=NEG_MASK,
                op0=op.is_equal,
                op1=op.mult,
            )

            hp_sbs = []
            for h, (hkT_sb, hkr_sb, mt, att) in enumerate(
                ((hk1T_sb, hk1r_sb, m1t, att1), (hk2T_sb, hk2r_sb, m2t, att2))
            ):
                m_u8 = mu8_pool.tile([128, MC, 128], U8, tag="mu8")
                nc.sync.dma_start(out=m_u8, in_=mt[b])

                ex = ex_pool.tile([128, M], F32, tag="ex")
                ssum = small.tile([128, 1], F32, tag="ssum")
                if variant == "prelu":
                    em = em_pool.tile([128, M], F32, tag="em")
                    for c in range(EC):
                        sl = slice(c * 512, (c + 1) * 512)
                        e_ps = psum_e.tile([128, 512], F32, tag="eps")
                        nc.tensor.matmul(
                            e_ps, lhsT=whT_sb, rhs=hkT_sb[:, sl],
                            start=True, stop=True,
                        )
                        el = eab_pool.tile([128, 512], F32, tag="el")
                        nc.scalar.activation(el, e_ps, AF.Prelu, alpha=ALPHA)
                        # mask-add on the idle GpSimd engine:
                        # leaky(e) + negoff is as masked as leaky(e + negoff)
                        nc.gpsimd.tensor_add(em[:, sl], el, negoff[:, sl])
                    # softmax without max-subtraction: exp(z) is safely finite
                    nc.scalar.activation(ex, em, AF.Exp, accum_out=ssum)
                else:  # dualexp
                    sums = small.tile([128, EC], F32, tag="sums")
                    for c in range(EC):
                        sl = slice(c * 512, (c + 1) * 512)
                        e_ps = psum_e.tile([128, 512], F32, tag="eps")
                        nc.tensor.matmul(
                            e_ps, lhsT=ident, rhs=negoff[:, sl], start=True, stop=False
                        )
                        nc.tensor.matmul(
                            e_ps, lhsT=whT_sb, rhs=hkT_sb[:, sl], start=False, stop=True
                        )
                        exa = eab_pool.tile([128, 512], F32, tag="exa")
                        nc.scalar.activation(exa, e_ps, AF.Exp)
                        exb = eab_pool.tile([128, 512], F32, tag="exb")
                        nc.scalar.activation(exb, e_ps, AF.Exp, scale=ALPHA)
                        # exp(leaky(z)) = max(exp(z), exp(alpha z)); accum = row sum
                        nc.vector.scalar_tensor_tensor(
                            out=ex[:, sl], in0=exa, scalar=0.0, op0=op.add,
                            op1=op.max, in1=exb, accum_out=sums[:, c : c + 1],
                        )
                    nc.vector.tensor_reduce(
                        ssum, sums, axis=mybir.AxisListType.X, op=op.add
                    )
                rinv = small.tile([128, 1], F32, tag="rinv")
                nc.vector.reciprocal(rinv, ssum)
                rinvk = small.tile([128, 1], F32, tag="rinvk")
                nc.vector.tensor_scalar_mul(rinvk, rinv, 1.0 / KEEP)

                hp_ps = psum_hp.tile([128, D], F32, tag="hp")
                for g in range(EC):
                    t_ps = psum_t.tile([128, 4, 128], F32, tag="tps")
                    for jj in range(4):
                        j = g * 4 + jj
                        nc.tensor.transpose(
                            t_ps[:, jj, :], ex[:, j * 128 : (j + 1) * 128], ident
                        )
                    attmT = attmT_pool.tile([128, 4, 128], BF16, tag="attmT")
                    nc.vector.tensor_mul(attmT, t_ps, m_u8[:, g * 4 : (g + 1) * 4, :])
                    for jj in range(4):
                        j = g * 4 + jj
                        nc.tensor.matmul(
                            hp_ps,
                            lhsT=attmT[:, jj, :],
                            rhs=hkr_sb[:, j, :],
                            start=(j == 0),
                            stop=(j == MC - 1),
                        )

                # att output = ex * (1/sum), in place, then store
                nc.vector.tensor_scalar_mul(ex, ex, rinv)
                nc.sync.dma_start(out=att[b * 128 : (b + 1) * 128, :], in_=ex)

                hp_sb = hp_pool.tile([128, D], F32, tag=f"hp{h}")
                nc.vector.tensor_scalar_mul(hp_sb, hp_ps, rinvk)
                hp_sbs.append(hp_sb)

            # out = elu(hp1 + hp2) = max(s,0) + exp(min(s,0)) - 1
            s = hp_pool.tile([128, D], F32, tag="hsum")
            nc.vector.tensor_add(s, hp_sbs[0], hp_sbs[1])
            sneg = hp_pool.tile([128, D], F32, tag="hneg")
            nc.vector.tensor_scalar_min(sneg, s, 0.0)
            spos = hp_pool.tile([128, D], F32, tag="hpos")
            nc.vector.tensor_scalar_max(spos, s, 0.0)
            ev = hp_pool.tile([128, D], F32, tag="hev")
            nc.scalar.activation(ev, sneg, AF.Exp)
            o = hp_pool.tile([128, D], F32, tag="hout")
            nc.vector.scalar_tensor_tensor(
                out=o, in0=ev, scalar=-1.0, op0=op.add, op1=op.add, in1=spos
            )
            nc.sync.dma_start(out=outp[b * 128 : (b + 1) * 128, :], in_=o)

    return nc


def _arrange_weight(a, dtype=np.float32):
    """[M, D] -> [128, M//128, D] with partition = m % 128 (within chunk)."""
    m, d = a.shape
    return np.ascontiguousarray(
        a.reshape(m // 128, 128, d).transpose(1, 0, 2).astype(dtype)
    )


def _arrange_T(rows, dtype):
    """[R, M] -> [R//128, 128, M//128, 128] with [b, p=m%128, c=m//128, r%128]."""
    r, m = rows.shape
    a = rows.reshape(r // 128, 128, m // 128, 128)  # [b, r, c, p]
    return np.ascontiguousarray(a.transpose(0, 3, 2, 1).astype(dtype))


def _arrange_maskT(mask_rows):
    return _arrange_T(mask_rows, np.uint8)


def _dropout_masks():
    """Reproduce the reference's bernoulli keep-masks ({0,1}) on host CPU."""
    import jax

    cpu = jax.devices("cpu")[0]
    with jax.default_device(cpu):
        dk1, dk2 = jax.random.split(jax.random.key(1234))
        m1 = jax.random.bernoulli(dk1, KEEP, (N_FULL, M_FULL))
        m2 = jax.random.bernoulli(dk2, KEEP, (N_FULL, M_FULL))
        return np.asarray(m1), np.asarray(m2)


_BUILT = {}


def _get_nc(R=N_FULL // N_CORES, M=M_FULL, D=D_FULL, variant="prelu"):
    key = (R, M, D, variant)
    if key not in _BUILT:
        nc = bacc.Bacc("TRN2", target_bir_lowering=False, debug=False,
                       num_devices=N_CORES)
        build_gat(nc, R, M, D, variant=variant)
        nc.compile()
        _BUILT[key] = nc
    return _BUILT[key]


def _make_in_maps(h_s, h_k1, h_k2, W):
    h_s = np.ascontiguousarray(h_s, dtype=np.float32)
    h_k1 = np.ascontiguousarray(h_k1, dtype=np.float32)
    h_k2 = np.ascontiguousarray(h_k2, dtype=np.float32)
    W = np.ascontiguousarray(W, dtype=np.float32)

    import ml_dtypes

    m1, m2 = _dropout_masks()
    R = N_FULL // N_CORES

    w_arr = _arrange_weight(W)
    hk1T = np.ascontiguousarray(h_k1.T)
    hk2T = np.ascontiguousarray(h_k2.T)
    hk1r = _arrange_weight(h_k1, ml_dtypes.bfloat16)
    hk2r = _arrange_weight(h_k2, ml_dtypes.bfloat16)

    in_maps = []
    for i in range(N_CORES):
        rows = slice(i * R, (i + 1) * R)
        hs_rows = h_s[rows]
        in_maps.append(
            {
                "hst": _arrange_T(hs_rows, np.float32),
                "m01": np.ascontiguousarray(
                    (hs_rows > 0).reshape(R // 128, 128, M_FULL).astype(np.uint8)
                ),
                "w": w_arr,
                "hk1t": hk1T,
                "hk2t": hk2T,
                "hk1r": hk1r,
                "hk2r": hk2r,
                "m1t": _arrange_maskT(m1[rows]),
                "m2t": _arrange_maskT(m2[rows]),
            }
        )
    return in_maps


def _gather(res):
    out = np.concatenate([r["outp"] for r in res], axis=0)
    att1 = np.concatenate([r["att1"] for r in res], axis=0)
    att2 = np.concatenate([r["att2"] for r in res], axis=0)
    return out, att1, att2


def kernel(h_s, h_k1, h_k2, W):
    from concourse.bass_utils import run_bass_kernel_spmd

    in_maps = _make_in_maps(h_s, h_k1, h_k2, W)
    nc = _get_nc()
    res = run_bass_kernel_spmd(nc, in_maps, list(range(N_CORES))).results
    return _gather(res)


# revision 29
# speedup vs baseline: 1.9531x; 1.0292x over previous
"""GAT layer (2 heads) Bass kernel for Trainium2, sharded over 8 NeuronCores.

Computation (per reference):
    Wh   = h_s @ W                      [N, D]
    e_h  = leaky_relu(Wh @ h_k^T)       [N, M]   (alpha = 0.2)
    att  = softmax(where(h_s > 0, e_h, -9e15), axis=-1)
    h'_h = (att * dropout_mask) @ h_k   [N, D]
    out  = elu(h'_1 + h'_2)
Returns (out, att1, att2).

Sharding: rows of h_s (N) split across 8 cores; weights replicated.
Dropout masks are reproduced on host (fixed jax threefry key) and shipped
as {0,1} uint8 in transposed layout; the 1/keep factor and the softmax
normalization are folded into a tiny per-row scale of h'.
"""

import sys
from contextlib import ExitStack

import numpy as np

if "/opt/trn_rl_repo" not in sys.path:
    sys.path.insert(0, "/opt/trn_rl_repo")

import concourse.bass as bass
import concourse.tile as tile
from concourse import bacc, mybir
from concourse.masks import make_identity

F32 = mybir.dt.float32
BF16 = mybir.dt.bfloat16
U8 = mybir.dt.uint8

N_FULL = 8192
M_FULL = 4096
D_FULL = 128
N_CORES = 8
ALPHA = 0.2
RATE = 0.1
KEEP = 1.0 - RATE
NEG_MASK = -1.0e30  # added to masked logits (reference uses -9e15; any << min kept logit works)
NEG_INIT = -3.0e38  # running-max init


def build_gat(nc, R, M, D, use_pool_cvt=False, variant="prelu", use_f32r=True):
    """Emit the per-core GAT kernel into `nc` (R rows per core).

    variant:
      "prelu"   — z = e + negoff on PE; ACT Prelu; one big ACT Exp with sum-accum.
      "dualexp" — exp(leaky(z)) = max(exp(z), exp(alpha*z)); DVE stt merges + sums.
    Both skip the softmax max-subtraction: |z| <= |Wh_r||hk_m| < 90 so exp stays
    finite in f32, and normalization divides the scale back out.
    """
    assert R % 128 == 0 and M % 512 == 0 and D == 128
    RB = R // 128   # row blocks
    MC = M // 128   # 128-wide m chunks
    EC = M // 512   # 512-wide e chunks

    op = mybir.AluOpType
    AF = mybir.ActivationFunctionType
    # fp32 matmuls run LOW+HIGH passes at 2 cyc/col (4x bf16 cost);
    # float32r is single-pass with tf32-class mantissa — fine for logits.
    MMDT = mybir.dt.float32r if use_f32r else F32

    hst = nc.dram_tensor("hst", [RB, 128, MC, 128], MMDT, kind="ExternalInput").ap()
    m01 = nc.dram_tensor("m01", [RB, 128, M], U8, kind="ExternalInput").ap()
    w = nc.dram_tensor("w", [128, MC, D], MMDT, kind="ExternalInput").ap()
    hk1T = nc.dram_tensor("hk1t", [D, M], MMDT, kind="ExternalInput").ap()
    hk2T = nc.dram_tensor("hk2t", [D, M], MMDT, kind="ExternalInput").ap()
    hk1r = nc.dram_tensor("hk1r", [128, MC, D], BF16, kind="ExternalInput").ap()
    hk2r = nc.dram_tensor("hk2r", [128, MC, D], BF16, kind="ExternalInput").ap()
    iden = nc.dram_tensor("iden", [128, 128], MMDT, kind="ExternalInput").ap()
    m1t = nc.dram_tensor("m1t", [RB, 128, MC, 128], U8, kind="ExternalInput").ap()
    m2t = nc.dram_tensor("m2t", [RB, 128, MC, 128], U8, kind="ExternalInput").ap()
    att1 = nc.dram_tensor("att1", [R, M], MMDT, kind="ExternalOutput").ap()
    att2 = nc.dram_tensor("att2", [R, M], MMDT, kind="ExternalOutput").ap()
    outp = nc.dram_tensor("outp", [R, D], F32, kind="ExternalOutput").ap()

    with tile.TileContext(nc) as tc, ExitStack() as ctx:
        const = ctx.enter_context(tc.tile_pool(name="const", bufs=1))
        hs_pool = ctx.enter_context(tc.tile_pool(name="hs", bufs=2))
        hsT_pool = ctx.enter_context(tc.tile_pool(name="hsT", bufs=3))
        negoff_pool = ctx.enter_context(tc.tile_pool(name="negoff", bufs=1))
        eab_pool = ctx.enter_context(tc.tile_pool(name="eab", bufs=3))
        em_pool = ctx.enter_context(tc.tile_pool(name="em", bufs=1))
        ex_pool = ctx.enter_context(tc.tile_pool(name="ex", bufs=2))
        mu8_pool = ctx.enter_context(tc.tile_pool(name="mu8", bufs=2))
        mf_pool = ctx.enter_context(tc.tile_pool(name="mf", bufs=3))
        attmT_pool = ctx.enter_context(tc.tile_pool(name="attmT", bufs=3))
        whT_pool = ctx.enter_context(tc.tile_pool(name="whT", bufs=2))
        small = ctx.enter_context(tc.tile_pool(name="small", bufs=4))
        hp_pool = ctx.enter_context(tc.tile_pool(name="hp", bufs=2))

        psum_t = ctx.enter_context(tc.tile_pool(name="psum_t", bufs=2, space="PSUM"))
        psum_wh = ctx.enter_context(tc.tile_pool(name="psum_wh", bufs=1, space="PSUM"))
        psum_e = ctx.enter_context(tc.tile_pool(name="psum_e", bufs=4, space="PSUM"))
        psum_hp = ctx.enter_context(tc.tile_pool(name="psum_hp", bufs=1, space="PSUM"))

        ident = const.tile([128, 128], MMDT)
        nc.sync.dma_start(out=ident, in_=iden)

        w_sb = const.tile([128, MC, D], MMDT)
        nc.sync.dma_start(out=w_sb, in_=w)
        hk1T_sb = const.tile([D, M], MMDT)
        nc.sync.dma_start(out=hk1T_sb, in_=hk1T)
        hk2T_sb = const.tile([D, M], MMDT)
        nc.sync.dma_start(out=hk2T_sb, in_=hk2T)
        hk1r_sb = const.tile([128, MC, D], BF16)
        nc.sync.dma_start(out=hk1r_sb, in_=hk1r)
        hk2r_sb = const.tile([128, MC, D], BF16)
        nc.sync.dma_start(out=hk2r_sb, in_=hk2r)

        for b in range(RB):
            # h_s^T tiles come pre-transposed from the host
            hsT_sb = hs_pool.tile([128, MC, 128], MMDT, tag="hst")
            nc.sync.dma_start(out=hsT_sb, in_=hst[b])
            m01_sb = mu8_pool.tile([128, M], U8, tag="m01")
            nc.sync.dma_start(out=m01_sb, in_=m01[b])

            # WhT[d, r] = sum_j W_j^T @ hsT_j
            wh_ps = psum_wh.tile([D, 128], F32, tag="wh")
            for j in range(MC):
                nc.tensor.matmul(
                    wh_ps,
                    lhsT=w_sb[:, j, :],
                    rhs=hsT_sb[:, j, :],
                    start=(j == 0),
                    stop=(j == MC - 1),
                )
            whT_sb = whT_pool.tile([D, 128], MMDT, tag="whT")
            nc.scalar.copy(whT_sb, wh_ps)

            # negoff = (h_s <= 0) * NEG_MASK   {0 kept, NEG masked}
            negoff = negoff_pool.tile([128, M], F32, tag="negoff")
            nc.vector.tensor_scalar(
                out=negoff,
                in0=m01_sb,
                scalar1=0,
                scalar2=NEG_MASK,
                op0=op.is_equal,
                op1=op.mult,
            )

            hp_sbs = []
            for h, (hkT_sb, hkr_sb, mt, att) in enumerate(
                ((hk1T_sb, hk1r_sb, m1t, att1), (hk2T_sb, hk2r_sb, m2t, att2))
            ):
                m_u8 = mu8_pool.tile([128, MC, 128], U8, tag="mu8")
                nc.sync.dma_start(out=m_u8, in_=mt[b])

                ex = ex_pool.tile([128, M], MMDT, tag="ex")
                ssum = small.tile([128, 1], F32, tag="ssum")
                if variant == "prelu":
                    em = em_pool.tile([128, M], F32, tag="em")
                    for c in range(EC):
                        sl = slice(c * 512, (c + 1) * 512)
                        e_ps = psum_e.tile([128, 512], F32, tag="eps")
                        nc.tensor.matmul(
                            e_ps, lhsT=whT_sb, rhs=hkT_sb[:, sl],
                            start=True, stop=True,
                        )
                        el = eab_pool.tile([128, 512], F32, tag="el")
                        nc.scalar.activation(el, e_ps, AF.Prelu, alpha=ALPHA)
                        # mask-add on the idle GpSimd engine:
                        # leaky(e) + negoff is as masked as leaky(e + negoff)
                        nc.gpsimd.tensor_add(em[:, sl], el, negoff[:, sl])
                    # softmax without max-subtraction: exp(z) is safely finite
                    nc.scalar.activation(ex, em, AF.Exp, accum_out=ssum)
                else:  # dualexp
                    sums = small.tile([128, EC], F32, tag="sums")
                    for c in range(EC):
                        sl = slice(c * 512, (c + 1) * 512)
                        e_ps = psum_e.tile([128, 512], F32, tag="eps")
                        nc.tensor.matmul(
                            e_ps, lhsT=ident, rhs=negoff[:, sl], start=True, stop=False
                        )
                        nc.tensor.matmul(
                            e_ps, lhsT=whT_sb, rhs=hkT_sb[:, sl], start=False, stop=True
                        )
                        exa = eab_pool.tile([128, 512], F32, tag="exa")
                        nc.scalar.activation(exa, e_ps, AF.Exp)
                        exb = eab_pool.tile([128, 512], F32, tag="exb")
                        nc.scalar.activation(exb, e_ps, AF.Exp, scale=ALPHA)
                        # exp(leaky(z)) = max(exp(z), exp(alpha z)); accum = row sum
                        nc.vector.scalar_tensor_tensor(
                            out=ex[:, sl], in0=exa, scalar=0.0, op0=op.add,
                            op1=op.max, in1=exb, accum_out=sums[:, c : c + 1],
                        )
                    nc.vector.tensor_reduce(
                        ssum, sums, axis=mybir.AxisListType.X, op=op.add
                    )
                rinv = small.tile([128, 1], F32, tag="rinv")
                nc.vector.reciprocal(rinv, ssum)
                rinvk = small.tile([128, 1], F32, tag="rinvk")
                nc.vector.tensor_scalar_mul(rinvk, rinv, 1.0 / KEEP)

                hp_ps = psum_hp.tile([128, D], F32, tag="hp")
                for g in range(EC):
                    t_ps = psum_t.tile([128, 4, 128], MMDT, tag="tps")
                    for jj in range(4):
                        j = g * 4 + jj
                        nc.tensor.transpose(
                            t_ps[:, jj, :], ex[:, j * 128 : (j + 1) * 128], ident
                        )
                    attmT = attmT_pool.tile([128, 4, 128], BF16, tag="attmT")
                    nc.vector.tensor_mul(attmT, t_ps, m_u8[:, g * 4 : (g + 1) * 4, :])
                    for jj in range(4):
                        j = g * 4 + jj
                        nc.tensor.matmul(
                            hp_ps,
                            lhsT=attmT[:, jj, :],
                            rhs=hkr_sb[:, j, :],
                            start=(j == 0),
                            stop=(j == MC - 1),
                        )

                # att output = ex * (1/sum), in place, then store
                nc.vector.tensor_scalar_mul(ex, ex, rinv)
                nc.sync.dma_start(out=att[b * 128 : (b + 1) * 128, :], in_=ex)

                hp_sb = hp_pool.tile([128, D], F32, tag=f"hp{h}")
                nc.vector.tensor_scalar_mul(hp_sb, hp_ps, rinvk)
                hp_sbs.append(hp_sb)

            # out = elu(hp1 + hp2) = max(s,0) + exp(min(s,0)) - 1
            s = hp_pool.tile([128, D], F32, tag="hsum")
            nc.vector.tensor_add(s, hp_sbs[0], hp_sbs[1])
            sneg = hp_pool.tile([128, D], F32, tag="hneg")
            nc.vector.tensor_scalar_min(sneg, s, 0.0)
            spos = hp_pool.tile([128, D], F32, tag="hpos")
            nc.vector.tensor_scalar_max(spos, s, 0.0)
            ev = hp_pool.tile([128, D], F32, tag="hev")
            nc.scalar.activation(ev, sneg, AF.Exp)
            o = hp_pool.tile([128, D], F32, tag="hout")
            nc.vector.scalar_tensor_tensor(
                out=o, in0=ev, scalar=-1.0, op0=op.add, op1=op.add, in1=spos
            )
            nc.sync.dma_start(out=outp[b * 128 : (b + 1) * 128, :], in_=o)

    return nc


def _arrange_weight(a, dtype=np.float32):
    """[M, D] -> [128, M//128, D] with partition = m % 128 (within chunk)."""
    m, d = a.shape
    return np.ascontiguousarray(
        a.reshape(m // 128, 128, d).transpose(1, 0, 2).astype(dtype)
    )


def _arrange_T(rows, dtype):
    """[R, M] -> [R//128, 128, M//128, 128] with [b, p=m%128, c=m//128, r%128]."""
    r, m = rows.shape
    a = rows.reshape(r // 128, 128, m // 128, 128)  # [b, r, c, p]
    return np.ascontiguousarray(a.transpose(0, 3, 2, 1).astype(dtype))


def _arrange_maskT(mask_rows):
    return _arrange_T(mask_rows, np.uint8)


def _dropout_masks():
    """Reproduce the reference's bernoulli keep-masks ({0,1}) on host CPU."""
    import jax

    cpu = jax.devices("cpu")[0]
    with jax.default_device(cpu):
        dk1, dk2 = jax.random.split(jax.random.key(1234))
        m1 = jax.random.bernoulli(dk1, KEEP, (N_FULL, M_FULL))
        m2 = jax.random.bernoulli(dk2, KEEP, (N_FULL, M_FULL))
        return np.asarray(m1), np.asarray(m2)


_BUILT = {}


def _get_nc(R=N_FULL // N_CORES, M=M_FULL, D=D_FULL, variant="prelu"):
    key = (R, M, D, variant)
    if key not in _BUILT:
        nc = bacc.Bacc("TRN2", target_bir_lowering=False, debug=False,
                       num_devices=N_CORES)
        build_gat(nc, R, M, D, variant=variant)
        nc.compile()
        _BUILT[key] = nc
    return _BUILT[key]


def _make_in_maps(h_s, h_k1, h_k2, W):
    h_s = np.ascontiguousarray(h_s, dtype=np.float32)
    h_k1 = np.ascontiguousarray(h_k1, dtype=np.float32)
    h_k2 = np.ascontiguousarray(h_k2, dtype=np.float32)
    W = np.ascontiguousarray(W, dtype=np.float32)

    import ml_dtypes

    m1, m2 = _dropout_masks()
    R = N_FULL // N_CORES

    w_arr = _arrange_weight(W)
    hk1T = np.ascontiguousarray(h_k1.T)
    hk2T = np.ascontiguousarray(h_k2.T)
    hk1r = _arrange_weight(h_k1, ml_dtypes.bfloat16)
    hk2r = _arrange_weight(h_k2, ml_dtypes.bfloat16)

    in_maps = []
    for i in range(N_CORES):
        rows = slice(i * R, (i + 1) * R)
        hs_rows = h_s[rows]
        in_maps.append(
            {
                "hst": _arrange_T(hs_rows, np.float32),
                "m01": np.ascontiguousarray(
                    (hs_rows > 0).reshape(R // 128, 128, M_FULL).astype(np.uint8)
                ),
                "iden": np.eye(128, dtype=np.float32),
                "w": w_arr,
                "hk1t": hk1T,
                "hk2t": hk2T,
                "hk1r": hk1r,
                "hk2r": hk2r,
                "m1t": _arrange_maskT(m1[rows]),
                "m2t": _arrange_maskT(m2[rows]),
            }
        )
    return in_maps


def _gather(res):
    out = np.concatenate([r["outp"] for r in res], axis=0)
    att1 = np.concatenate([r["att1"] for r in res], axis=0)
    att2 = np.concatenate([r["att2"] for r in res], axis=0)
    return out, att1, att2


def kernel(h_s, h_k1, h_k2, W):
    from concourse.bass_utils import run_bass_kernel_spmd

    in_maps = _make_in_maps(h_s, h_k1, h_k2, W)
    nc = _get_nc()
    res = run_bass_kernel_spmd(nc, in_maps, list(range(N_CORES))).results
    return _gather(res)
